# revision 1
# baseline (speedup 1.0000x reference)
"""Fused cross-attention kernel for Trainium2, 8-way data-parallel over batch.

Per core (one batch element):
  QT[d, hw] = (Wq @ Jp + bq)        via lhsT=[WqT; bq], rhs=[Jp; 1]
  K [d, hw] = (Wk @ Jg + bk)
  V [hw, d] = (Jg.T @ WvT + bv)     stored with 2 ones cols -> [V | 1 | 1]
  For each q-block (512 queries):
    for each k-chunk (128 keys):
      S^T[k, q]   = K-chunk.T @ QT          (PSUM, 2 matmuls over d-halves)
      E^T         = exp(S^T / 16)           (scalar engine, PSUM -> SBUF)
      O[q, 258]  += E^T-subtile.T @ [V|1|1] (PSUM accum; col 256 = softmax sum)
    out[q, d] = O[:, :256] * (1 / O[:, 256])

All matmuls run in float32r (TF32-like full-rate fp32 mode). fp32r ISA
restrictions honored: even innermost dst size, dst start_partition 0, inputs
materialized as float32r by their producers (DRAM tensors declared float32r).
Q/K/V live in per-block tiles so attention can overlap the projection tail.
"""

import sys

sys.path.insert(0, "/opt/trn_rl_repo")

import numpy as np

import concourse.bacc as bacc
import concourse.mybir as mybir
import concourse.tile as tile
from concourse.bass_utils import run_bass_kernel_spmd

B, C, H, W = 8, 64, 64, 64
HW = H * W  # 4096
D = 256
CE = C + 1  # channels + ones row for bias folding
N_CORES = 8
QB = 512  # queries per block
N_QB = HW // QB  # 8
N_KC = HW // 128  # 32 key chunks
DV = D + 2  # V row width: 256 values + 2 ones columns (fp32r needs even dst)
F32 = mybir.dt.float32
MM_DT = mybir.dt.float32r  # full-rate fp32 matmul mode (TF32-like)

_CACHE = {}


def build_module(
    reps: int = 1,
    st_bufs: int = 3,
    op_bufs: int = 5,
    ep_bufs: int = 3,
    split: int = 1,
    pp_bufs: int = 4,
    dtype: str = "f32r",
    exp_split: int = 1,
    ck_pair: bool = False,
):
    mm_dt = MM_DT if dtype == "f32r" else mybir.dt.bfloat16
    nc = bacc.Bacc("TRN2", target_bir_lowering=False)
    jp_d = nc.dram_tensor("jp", [CE, HW], mm_dt, kind="ExternalInput")
    jg_d = nc.dram_tensor("jg", [CE, HW], mm_dt, kind="ExternalInput")
    wq_d = nc.dram_tensor("wq", [CE, D], mm_dt, kind="ExternalInput")
    wk_d = nc.dram_tensor("wk", [CE, D], mm_dt, kind="ExternalInput")
    wv_d = nc.dram_tensor("wv", [CE, D], mm_dt, kind="ExternalInput")
    ones_d = nc.dram_tensor("ones", [128, N_KC, 2], mm_dt, kind="ExternalInput")
    out_d = nc.dram_tensor("out", [HW, D], F32, kind="ExternalOutput")

    with tile.TileContext(nc) as tc:
        with tc.tile_pool(name="const", bufs=1) as const:
            jp_t = const.tile([CE, HW], mm_dt, tag="jp")
            jg_t = const.tile([CE, HW], mm_dt, tag="jg")
            wq_t = const.tile([CE, D], mm_dt, tag="wq")
            wk_t = const.tile([CE, D], mm_dt, tag="wk")
            wv_t = const.tile([CE, D], mm_dt, tag="wv")
            # per-block tiles for fine-grained dependencies
            qt_b = [
                const.tile([128, 2, QB], mm_dt, tag=f"qt{g}", name=f"qt_{g}")
                for g in range(N_QB)
            ]
            kt_g = [
                const.tile([128, 2, QB], mm_dt, tag=f"kt{g}", name=f"kt_{g}")
                for g in range(N_QB)
            ]
            vt_g = [
                const.tile([128, 4, DV], mm_dt, tag=f"vt{g}", name=f"vt_{g}")
                for g in range(N_QB)
            ]

            nc.sync.dma_start(wq_t[:], wq_d[:])
            nc.sync.dma_start(wk_t[:], wk_d[:])
            nc.sync.dma_start(wv_t[:], wv_d[:])
            for g in range(N_QB):
                hs = slice(g * QB, (g + 1) * QB)
                nc.sync.dma_start(jg_t[:, hs], jg_d[:, hs])
                nc.sync.dma_start(jp_t[:, hs], jp_d[:, hs])
                nc.sync.dma_start(vt_g[g][:, :, D:DV], ones_d[:, 4 * g : 4 * g + 4, :])

            for _rep in range(reps):
                # ---- projections ----
                # Order: Q(0) first (attention qb=0 needs it), then K/V in
                # ascending k-chunk order so attention consumes them streaming,
                # remaining Q blocks at the end.
                with tc.tile_pool(name="pp", bufs=pp_bufs, space="PSUM") as pp:

                    def proj_q(g):
                        hs = slice(g * QB, (g + 1) * QB)
                        for dh in range(2):
                            ds = slice(dh * 128, (dh + 1) * 128)
                            psq = pp.tile([128, QB], F32, tag="proj")
                            nc.tensor.matmul(psq[:], wq_t[:, ds], jp_t[:, hs])
                            nc.vector.tensor_copy(qt_b[g][:, dh, :], psq[:])

                    proj_q(0)
                    for g in range(N_QB):
                        hs = slice(g * QB, (g + 1) * QB)
                        for dh in range(2):
                            ds = slice(dh * 128, (dh + 1) * 128)
                            psk = pp.tile([128, QB], F32, tag="proj")
                            nc.tensor.matmul(psk[:], wk_t[:, ds], jg_t[:, hs])
                            nc.vector.tensor_copy(kt_g[g][:, dh, :], psk[:])
                        for j in range(4):
                            ck = 4 * g + j
                            ks = slice(ck * 128, (ck + 1) * 128)
                            psv = pp.tile([128, D], F32, tag="projv")
                            nc.tensor.matmul(psv[:], jg_t[:, ks], wv_t[:])
                            nc.vector.tensor_copy(vt_g[g][:, j, :D], psv[:])
                    for g in range(1, N_QB):
                        proj_q(g)

                # ---- attention ----
                SW = QB // split  # S^T / exp tile width
                QS_PER = SW // 128  # q-subtiles per exp tile
                with (
                    tc.tile_pool(name="stp", bufs=st_bufs, space="PSUM") as stp,
                    tc.tile_pool(name="op", bufs=op_bufs, space="PSUM") as op,
                    tc.tile_pool(name="ep", bufs=ep_bufs) as ep,
                    tc.tile_pool(name="outp", bufs=3) as outp,
                    tc.tile_pool(name="lp", bufs=4) as lp,
                ):
                    for qb in range(N_QB):
                        o_ps = [
                            op.tile([128, DV], F32, tag="o", name=f"o_{qb}_{i}")
                            for i in range(4)
                        ]
                        if ck_pair:
                            for cp in range(N_KC // 2):
                                st2 = stp.tile([128, 2, QB], F32, tag="st")
                                for c in range(2):
                                    ck = 2 * cp + c
                                    g, j = ck // 4, ck % 4
                                    for dh in range(2):
                                        nc.tensor.matmul(
                                            st2[:, c, :],
                                            kt_g[g][:, dh, j * 128 : (j + 1) * 128],
                                            qt_b[qb][:, dh, :],
                                            start=(dh == 0),
                                            stop=(dh == 1),
                                        )
                                et2 = ep.tile([128, 2 * QB], mm_dt, tag="e")
                                nc.scalar.activation(
                                    et2[:],
                                    st2[:, :, :],
                                    mybir.ActivationFunctionType.Exp,
                                    scale=1.0 / 16.0,
                                )
                                for c in range(2):
                                    ck = 2 * cp + c
                                    g, j = ck // 4, ck % 4
                                    for i in range(4):
                                        nc.tensor.matmul(
                                            o_ps[i][:],
                                            et2[
                                                :,
                                                c * QB + i * 128 : c * QB + (i + 1) * 128,
                                            ],
                                            vt_g[g][:, j, :],
                                            start=(ck == 0),
                                            stop=(ck == N_KC - 1),
                                        )
                            for qsub in range(4):
                                row = qb * 4 + qsub
                                linv = lp.tile([128, 1], F32, tag="l")
                                nc.vector.reciprocal(linv[:], o_ps[qsub][:, D : D + 1])
                                ot = outp.tile([128, D], F32, tag="ot")
                                nc.vector.tensor_scalar_mul(
                                    ot[:], o_ps[qsub][:, :D], linv[:]
                                )
                                nc.sync.dma_start(
                                    out_d[row * 128 : (row + 1) * 128, :], ot[:]
                                )
                            continue
                        for ck in range(N_KC):
                            g, j = ck // 4, ck % 4
                            for sp in range(split):
                                qlo = sp * SW
                                st = stp.tile([128, SW], F32, tag="st")
                                for dh in range(2):
                                    nc.tensor.matmul(
                                        st[:],
                                        kt_g[g][:, dh, j * 128 : (j + 1) * 128],
                                        qt_b[qb][:, dh, qlo : qlo + SW],
                                        start=(dh == 0),
                                        stop=(dh == 1),
                                    )
                                EW = SW // exp_split
                                EQ = EW // 128
                                for es in range(exp_split):
                                    et = ep.tile([128, EW], mm_dt, tag="e")
                                    nc.scalar.activation(
                                        et[:],
                                        st[:, es * EW : (es + 1) * EW],
                                        mybir.ActivationFunctionType.Exp,
                                        scale=1.0 / 16.0,
                                    )
                                    for i in range(EQ):
                                        qsub = sp * QS_PER + es * EQ + i
                                        nc.tensor.matmul(
                                            o_ps[qsub][:],
                                            et[:, i * 128 : (i + 1) * 128],
                                            vt_g[g][:, j, :],
                                            start=(ck == 0),
                                            stop=(ck == N_KC - 1),
                                        )
                        for qsub in range(4):
                            row = qb * 4 + qsub
                            linv = lp.tile([128, 1], F32, tag="l")
                            nc.vector.reciprocal(linv[:], o_ps[qsub][:, D : D + 1])
                            ot = outp.tile([128, D], F32, tag="ot")
                            nc.vector.tensor_scalar_mul(
                                ot[:], o_ps[qsub][:, :D], linv[:]
                            )
                            nc.sync.dma_start(
                                out_d[row * 128 : (row + 1) * 128, :], ot[:]
                            )

    nc.compile()
    return nc


def _get_module(reps: int = 1, **kw):
    key = (reps, tuple(sorted(kw.items())))
    if key not in _CACHE:
        _CACHE[key] = build_module(reps, **kw)
    return _CACHE[key]


_ONES = np.ones((128, N_KC, 2), np.float32)
_ROW1 = np.ones((1, HW), np.float32)


def _prep_in_maps(inputs, dtype="f32r"):
    import ml_dtypes

    npdt = np.float32 if dtype == "f32r" else ml_dtypes.bfloat16
    jp = np.asarray(inputs["Jp_embedding"], np.float32).reshape(B, C, HW)
    jg = np.asarray(inputs["Jg_embedding"], np.float32).reshape(B, C, HW)
    wq = np.concatenate(
        [
            np.asarray(inputs["Wq"], np.float32).T,
            np.asarray(inputs["bq"], np.float32)[None, :],
        ],
        0,
    )
    wk = np.concatenate(
        [
            np.asarray(inputs["Wk"], np.float32).T,
            np.asarray(inputs["bk"], np.float32)[None, :],
        ],
        0,
    )
    wv = np.concatenate(
        [
            np.asarray(inputs["Wv"], np.float32).T,
            np.asarray(inputs["bv"], np.float32)[None, :],
        ],
        0,
    )
    return [
        {
            "jp": np.concatenate([jp[b], _ROW1], 0).astype(npdt),
            "jg": np.concatenate([jg[b], _ROW1], 0).astype(npdt),
            "wq": wq.astype(npdt),
            "wk": wk.astype(npdt),
            "wv": wv.astype(npdt),
            "ones": _ONES.astype(npdt),
        }
        for b in range(B)
    ]


def kernel(**inputs):
    nc = _get_module()
    in_maps = _prep_in_maps(inputs)
    res = run_bass_kernel_spmd(nc, in_maps, core_ids=list(range(N_CORES)))
    return np.stack(
        [res.results[b]["out"].reshape(D, H, W) for b in range(B)], axis=0
    )



# revision 4
# speedup vs baseline: 1.3873x; 1.3873x over previous
"""Fused cross-attention kernel for Trainium2, 8-way data-parallel over batch.

Low-rank decomposition: S = Q^T K = Jp'^T (Wq' Wk'^T) Jg' = Jp'^T G where
G = M Jg' is computed on host ([65, HW], M = Wq' Wk'^T is 65x65). This cuts
the score matmul contraction from 256 (d) to 65 (c), halving PE work, and
eliminates the Q/K projections entirely.

Value side uses the same trick: O = P V = P (Jg'^T Wv') = (P Jg'^T) Wv' = T Wv'
where T = P Jg'^T is [q, 66] (col 64 = softmax denominator via Jg' ones row,
col 65 = zero pad). Accumulating T costs 66 output columns per k-chunk instead
of 258 for direct P V.

Per core (one batch element):
  for each q-block (512 queries):
    for each k-chunk-pair (2 x 128 keys):
      S^T[k, 2, q] = G-chunk^T Jp'         (PE, bf16, PSUM 2 banks)
      E^T = exp(S^T / 16)                  (ScalarE, one 1024-wide activation)
      T[q, 4, 66] += E^T-subtile^T Jgt     (PE, bf16, 8 x 66-col matmuls)
    T -> SBUF, transpose via PE identity, O[q, 256] = T^T^T Wv'  (PE)
    out[q, :] = O * (1 / T[q, 64])         (DVE)

All matmul operands bf16 (1 cycle/col on PE regardless of width); PSUM
accumulation is fp32. ScalarE (exp) is the pacing engine at ~1us per
1024-element activation; PE runs at ~65% occupancy underneath it.
"""

import sys

sys.path.insert(0, "/opt/trn_rl_repo")

import numpy as np

import concourse.bacc as bacc
import concourse.mybir as mybir
import concourse.tile as tile
from concourse.bass_utils import run_bass_kernel_spmd

B, C, H, W = 8, 64, 64, 64
HW = H * W  # 4096
D = 256
CE = C + 1  # channels + ones row (bias folding)
CT = CE + 1  # T width: 65 channels + zero pad -> 66
N_CORES = 8
QB = 512  # queries per block
N_QB = HW // QB  # 8
N_KC = HW // 128  # 32 key chunks
N_CP = N_KC // 2  # 16 key-chunk pairs
F32 = mybir.dt.float32
BF16 = mybir.dt.bfloat16

_CACHE = {}


def build_module(reps: int = 1, st_bufs: int = 2, ep_bufs: int = 3, wide_exp: bool = True):
    nc = bacc.Bacc("TRN2", target_bir_lowering=False)
    jp_d = nc.dram_tensor("jp", [CE, N_QB, QB], BF16, kind="ExternalInput")
    g_d = nc.dram_tensor("g", [CE, N_KC, 128], BF16, kind="ExternalInput")
    jgt_d = nc.dram_tensor("jgt", [128, N_KC, CT], BF16, kind="ExternalInput")
    wv_d = nc.dram_tensor("wv", [CT, D], BF16, kind="ExternalInput")
    id_d = nc.dram_tensor("ident", [128, 128], F32, kind="ExternalInput")
    out_d = nc.dram_tensor("out", [HW, D], F32, kind="ExternalOutput")

    with tile.TileContext(nc) as tc:
        with tc.tile_pool(name="const", bufs=1) as const:
            jp_t = const.tile([CE, N_QB, QB], BF16, tag="jp")
            g_t = const.tile([CE, N_KC, 128], BF16, tag="g")
            jgt_t = const.tile([128, N_KC, CT], BF16, tag="jgt")
            wv_t = const.tile([CT, D], BF16, tag="wv")
            id_t = const.tile([128, 128], F32, tag="ident")

            nc.sync.dma_start(id_t[:], id_d[:])
            nc.sync.dma_start(wv_t[:], wv_d[:])
            # qb=0 inputs first so attention starts early
            nc.sync.dma_start(jp_t[:, 0, :], jp_d[:, 0, :])
            for cq in range(4):
                ks = slice(8 * cq, 8 * cq + 8)
                nc.sync.dma_start(g_t[:, ks, :], g_d[:, ks, :])
            for cq in range(4):
                ks = slice(8 * cq, 8 * cq + 8)
                nc.sync.dma_start(jgt_t[:, ks, :], jgt_d[:, ks, :])
            nc.sync.dma_start(jp_t[:, 1:, :], jp_d[:, 1:, :])

            for _rep in range(reps):
                with (
                    tc.tile_pool(name="stp", bufs=st_bufs, space="PSUM") as stp,
                    tc.tile_pool(name="tp", bufs=2, space="PSUM") as tp,
                    tc.tile_pool(name="ttp", bufs=1, space="PSUM") as ttp,
                    tc.tile_pool(name="op", bufs=1, space="PSUM") as op,
                    tc.tile_pool(name="ep", bufs=ep_bufs) as ep,
                    tc.tile_pool(name="tsp", bufs=2) as tsp,
                    tc.tile_pool(name="ttsp", bufs=2) as ttsp,
                    tc.tile_pool(name="outp", bufs=3) as outp,
                    tc.tile_pool(name="lp", bufs=4) as lp,
                ):
                    for qb in range(N_QB):
                        t_ps = tp.tile([128, 4, CT], F32, tag="t", name=f"t_{qb}")
                        for cp in range(N_CP):
                            st2 = stp.tile([128, 2, QB], F32, tag="st")
                            for c in range(2):
                                kc = 2 * cp + c
                                nc.tensor.matmul(
                                    st2[:, c, :],
                                    g_t[:, kc, :],
                                    jp_t[:, qb, :],
                                )
                            if wide_exp:
                                et = ep.tile([128, 2, QB], BF16, tag="e")
                                nc.scalar.activation(
                                    et[:, :, :],
                                    st2[:, :, :],
                                    mybir.ActivationFunctionType.Exp,
                                    scale=1.0 / 16.0,
                                )
                            else:
                                et = ep.tile([128, 2, QB], BF16, tag="e")
                                for c in range(2):
                                    nc.scalar.activation(
                                        et[:, c, :],
                                        st2[:, c, :],
                                        mybir.ActivationFunctionType.Exp,
                                        scale=1.0 / 16.0,
                                    )
                            for c in range(2):
                                kc = 2 * cp + c
                                for j in range(4):
                                    # NOTE: start=True resets the WHOLE PSUM
                                    # bank, so only the first matmul of the
                                    # first group may use it — its reset
                                    # zero-fills the other 3 groups' regions.
                                    nc.tensor.matmul(
                                        t_ps[:, j, :],
                                        et[:, c, j * 128 : (j + 1) * 128],
                                        jgt_t[:, kc, :],
                                        start=(kc == 0 and j == 0),
                                        stop=(kc == N_KC - 1),
                                        skip_group_check=True,
                                    )
                        # epilogue: transpose T and project through Wv'
                        t_sb = tsp.tile([128, 4, CT], F32, tag="ts", name=f"ts_{qb}")
                        nc.vector.tensor_copy(t_sb[:], t_ps[:])
                        tt_ps = ttp.tile([128, 4, 128], F32, tag="tt", name=f"tt_{qb}")
                        for j in range(4):
                            nc.tensor.matmul(
                                tt_ps[:CT, j, :],
                                t_sb[:, j, :],
                                id_t[:],
                                is_transpose=True,
                                start=(j == 0),
                                stop=(j == 3),
                                skip_group_check=True,
                            )
                        tt_sb = ttsp.tile([128, 4, 128], BF16, tag="tts", name=f"tts_{qb}")
                        nc.vector.tensor_copy(tt_sb[:CT, :, :], tt_ps[:CT, :, :])
                        for j in range(4):
                            row = qb * 4 + j
                            o_ps = op.tile([128, D], F32, tag="o")
                            nc.tensor.matmul(
                                o_ps[:], tt_sb[:CT, j, :], wv_t[:]
                            )
                            linv = lp.tile([128, 1], F32, tag="l")
                            nc.vector.reciprocal(linv[:], t_sb[:, j, C : C + 1])
                            ot = outp.tile([128, D], F32, tag="ot")
                            nc.vector.tensor_scalar_mul(ot[:], o_ps[:], linv[:])
                            nc.sync.dma_start(
                                out_d[row * 128 : (row + 1) * 128, :], ot[:]
                            )

    nc.compile()
    return nc


def _get_module(reps: int = 1, **kw):
    key = (reps, tuple(sorted(kw.items())))
    if key not in _CACHE:
        _CACHE[key] = build_module(reps, **kw)
    return _CACHE[key]


_ROW1 = np.ones((1, HW), np.float32)
_EYE = np.eye(128, dtype=np.float32)


def _prep_in_maps(inputs):
    import ml_dtypes

    bf = ml_dtypes.bfloat16
    jp = np.asarray(inputs["Jp_embedding"], np.float32).reshape(B, C, HW)
    jg = np.asarray(inputs["Jg_embedding"], np.float32).reshape(B, C, HW)
    wq = np.concatenate(
        [np.asarray(inputs["Wq"], np.float32).T, np.asarray(inputs["bq"], np.float32)[None, :]], 0
    )
    wk = np.concatenate(
        [np.asarray(inputs["Wk"], np.float32).T, np.asarray(inputs["bk"], np.float32)[None, :]], 0
    )
    wv = np.concatenate(
        [
            np.asarray(inputs["Wv"], np.float32).T,
            np.asarray(inputs["bv"], np.float32)[None, :],
            np.zeros((1, D), np.float32),
        ],
        0,
    )
    m = wq @ wk.T  # [65, 65]
    ident = _EYE
    wv_b = wv.astype(bf)
    maps = []
    for b in range(B):
        jp_b = np.concatenate([jp[b], _ROW1], 0)  # [65, HW]
        jg_b = np.concatenate([jg[b], _ROW1], 0)
        g_b = m @ jg_b  # [65, HW]
        jgt_b = np.concatenate([jg_b.T, np.zeros((HW, 1), np.float32)], 1)  # [HW, 66]
        maps.append(
            {
                "jp": jp_b.reshape(CE, N_QB, QB).astype(bf),
                "g": g_b.reshape(CE, N_KC, 128).astype(bf),
                "jgt": np.ascontiguousarray(
                    jgt_b.reshape(N_KC, 128, CT).transpose(1, 0, 2)
                ).astype(bf),
                "wv": wv_b,
                "ident": ident,
            }
        )
    return maps


def kernel(**inputs):
    nc = _get_module()
    in_maps = _prep_in_maps(inputs)
    res = run_bass_kernel_spmd(nc, in_maps, core_ids=list(range(N_CORES)))
    return np.stack(
        [res.results[b]["out"].reshape(D, H, W) for b in range(B)], axis=0
    )


# revision 7
# speedup vs baseline: 1.8011x; 1.2983x over previous
"""Fused cross-attention kernel for Trainium2, 8-way data-parallel over batch.

Low-rank decomposition: S = Q^T K = Jp'^T (Wq' Wk'^T) Jg' = Jp'^T G where
G = M Jg' is computed on host ([65, HW], M = Wq' Wk'^T is 65x65). This cuts
the score matmul contraction from 256 (d) to 65 (c), halving PE work, and
eliminates the Q/K projections entirely.

Value side uses the same trick: O = P V = P (Jg'^T Wv') = (P Jg'^T) Wv' = T Wv'
where T = P Jg'^T is [q, 66] (col 64 = softmax denominator via Jg' ones row,
col 65 = zero pad). Accumulating T costs 66 output columns per k-chunk instead
of 258 for direct P V.

Per core (one batch element):
  for each q-block (512 queries):
    for each k-chunk-pair (2 x 128 keys):
      S^T[k, 2, q] = G-chunk^T Jp'         (PE, bf16, PSUM 2 banks)
      E^T = exp(S^T / 16)                  (ScalarE, one 1024-wide activation)
      T[q, 4, 66] += E^T-subtile^T Jgt     (PE, bf16, 8 x 66-col matmuls)
    T -> SBUF, transpose via PE identity, O[q, 256] = T^T^T Wv'  (PE)
    out[q, :] = O * (1 / T[q, 64])         (DVE)

All matmul operands bf16 (1 cycle/col on PE regardless of width); PSUM
accumulation is fp32. ScalarE (exp) is the pacing engine at ~1us per
1024-element activation; PE runs at ~65% occupancy underneath it.
"""

import sys

sys.path.insert(0, "/opt/trn_rl_repo")

import numpy as np

import concourse.bacc as bacc
import concourse.mybir as mybir
import concourse.tile as tile
from concourse.bass_utils import run_bass_kernel_spmd

B, C, H, W = 8, 64, 64, 64
HW = H * W  # 4096
D = 256
CE = C + 1  # channels + ones row (bias folding)
CT = CE + 1  # T width: 65 channels + zero pad -> 66
N_CORES = 8
QB = 512  # queries per block
N_QB = HW // QB  # 8
N_KC = HW // 128  # 32 key chunks
N_CP = N_KC // 2  # 16 key-chunk pairs
F32 = mybir.dt.float32
BF16 = mybir.dt.bfloat16

_CACHE = {}


GROUPS = [3, 3, 3, 3, 3, 3, 3, 3, 3, 3, 2]  # k-chunks per exp activation (sum 32)


def build_module(reps: int = 1, st_bufs: int = 2, ep_bufs: int = 4, groups=None):
    if groups is None:
        groups = GROUPS
    assert sum(groups) == N_KC
    gmax = max(groups)
    nc = bacc.Bacc("TRN2", target_bir_lowering=False)
    jp_d = nc.dram_tensor("jp", [CE, N_QB, QB], BF16, kind="ExternalInput")
    g_d = nc.dram_tensor("g", [CE, N_KC, 128], BF16, kind="ExternalInput")
    jgt_d = nc.dram_tensor("jgt", [128, N_KC, CT], BF16, kind="ExternalInput")
    wv_d = nc.dram_tensor("wv", [CT, D], BF16, kind="ExternalInput")
    id_d = nc.dram_tensor("ident", [128, 128], F32, kind="ExternalInput")
    out_d = nc.dram_tensor("out", [HW, D], F32, kind="ExternalOutput")

    with tile.TileContext(nc) as tc:
        with tc.tile_pool(name="const", bufs=1) as const:
            jp_t = const.tile([CE, N_QB, QB], BF16, tag="jp")
            g_t = const.tile([CE, N_KC, 128], BF16, tag="g")
            jgt_t = const.tile([128, N_KC, CT], BF16, tag="jgt")
            wv_t = const.tile([CT, D], BF16, tag="wv")
            id_t = const.tile([128, 128], F32, tag="ident")

            # tiny dummy exp issued first: forces the ACT table load to happen
            # during the input-DMA window instead of on the critical path
            dummy = const.tile([1, 2], F32, tag="dummy")
            nc.vector.memset(dummy[:], 0.0)
            nc.scalar.activation(
                dummy[:], dummy[:], mybir.ActivationFunctionType.Exp
            )

            nc.sync.dma_start(id_t[:], id_d[:])
            nc.sync.dma_start(wv_t[:], wv_d[:])
            # qb=0 inputs first so attention starts early
            nc.sync.dma_start(jp_t[:, 0, :], jp_d[:, 0, :])
            for cq in range(4):
                ks = slice(8 * cq, 8 * cq + 8)
                nc.sync.dma_start(g_t[:, ks, :], g_d[:, ks, :])
            for cq in range(4):
                ks = slice(8 * cq, 8 * cq + 8)
                nc.sync.dma_start(jgt_t[:, ks, :], jgt_d[:, ks, :])
            nc.sync.dma_start(jp_t[:, 1:, :], jp_d[:, 1:, :])

            for _rep in range(reps):
                with (
                    tc.tile_pool(name="stp", bufs=st_bufs, space="PSUM") as stp,
                    tc.tile_pool(name="tp", bufs=1, space="PSUM") as tp,
                    tc.tile_pool(name="mx", bufs=1, space="PSUM") as mx,
                    tc.tile_pool(name="ep", bufs=ep_bufs) as ep,
                    tc.tile_pool(name="tsp", bufs=2) as tsp,
                    tc.tile_pool(name="ttsp", bufs=2) as ttsp,
                    tc.tile_pool(name="outp", bufs=3) as outp,
                    tc.tile_pool(name="lp", bufs=4) as lp,
                ):
                    for qb in range(N_QB):
                        t_ps = tp.tile([128, 4, CT], F32, tag="t", name=f"t_{qb}")
                        kc0 = 0
                        for gi, gn in enumerate(groups):
                            st = stp.tile(
                                [128, gmax, QB], F32, tag="st", name=f"st_{qb}_{gi}"
                            )
                            for c in range(gn):
                                nc.tensor.matmul(
                                    st[:, c, :],
                                    g_t[:, kc0 + c, :],
                                    jp_t[:, qb, :],
                                )
                            et = ep.tile([128, gmax, QB], BF16, tag="e")
                            nc.scalar.activation(
                                et[:, :gn, :],
                                st[:, :gn, :],
                                mybir.ActivationFunctionType.Exp,
                                scale=1.0 / 16.0,
                            )
                            for c in range(gn):
                                kc = kc0 + c
                                for j in range(4):
                                    # NOTE: start=True resets the WHOLE PSUM
                                    # bank, so only the first matmul of the
                                    # first group may use it — its reset
                                    # zero-fills the other 3 groups' regions.
                                    nc.tensor.matmul(
                                        t_ps[:, j, :],
                                        et[:, c, j * 128 : (j + 1) * 128],
                                        jgt_t[:, kc, :],
                                        start=(kc == 0 and j == 0),
                                        stop=(kc == N_KC - 1),
                                        skip_group_check=True,
                                    )
                            kc0 += gn
                        # epilogue: transpose T and project through Wv'
                        t_sb = tsp.tile([128, 4, CT], F32, tag="ts", name=f"ts_{qb}")
                        nc.vector.tensor_copy(t_sb[:], t_ps[:])
                        tt_ps = mx.tile(
                            [128, 4, 128], F32, tag="x", name=f"tt_{qb}"
                        )
                        for j in range(4):
                            nc.tensor.matmul(
                                tt_ps[:CT, j, :],
                                t_sb[:, j, :],
                                id_t[:],
                                is_transpose=True,
                                start=(j == 0),
                                stop=(j == 3),
                                skip_group_check=True,
                            )
                        tt_sb = ttsp.tile([128, 4, 128], BF16, tag="tts", name=f"tts_{qb}")
                        nc.vector.tensor_copy(tt_sb[:CT, :, :], tt_ps[:CT, :, :])
                        for j in range(4):
                            row = qb * 4 + j
                            o_ps = mx.tile(
                                [128, D],
                                F32,
                                tag="x",
                                name=f"o_{qb}_{j}",
                                padded_shape=[128, 4 * 128],
                            )
                            nc.tensor.matmul(
                                o_ps[:], tt_sb[:CT, j, :], wv_t[:]
                            )
                            linv = lp.tile([128, 1], F32, tag="l")
                            nc.vector.reciprocal(linv[:], t_sb[:, j, C : C + 1])
                            ot = outp.tile([128, D], F32, tag="ot")
                            nc.vector.tensor_scalar_mul(ot[:], o_ps[:], linv[:])
                            nc.sync.dma_start(
                                out_d[row * 128 : (row + 1) * 128, :], ot[:]
                            )

    nc.compile()
    return nc


def _get_module(reps: int = 1, **kw):
    key = (reps, tuple(sorted(kw.items())))
    if key not in _CACHE:
        _CACHE[key] = build_module(reps, **kw)
    return _CACHE[key]


_ROW1 = np.ones((1, HW), np.float32)
_EYE = np.eye(128, dtype=np.float32)


def _prep_in_maps(inputs):
    import ml_dtypes

    bf = ml_dtypes.bfloat16
    jp = np.asarray(inputs["Jp_embedding"], np.float32).reshape(B, C, HW)
    jg = np.asarray(inputs["Jg_embedding"], np.float32).reshape(B, C, HW)
    wq = np.concatenate(
        [np.asarray(inputs["Wq"], np.float32).T, np.asarray(inputs["bq"], np.float32)[None, :]], 0
    )
    wk = np.concatenate(
        [np.asarray(inputs["Wk"], np.float32).T, np.asarray(inputs["bk"], np.float32)[None, :]], 0
    )
    wv = np.concatenate(
        [
            np.asarray(inputs["Wv"], np.float32).T,
            np.asarray(inputs["bv"], np.float32)[None, :],
            np.zeros((1, D), np.float32),
        ],
        0,
    )
    m = wq @ wk.T  # [65, 65]
    ident = _EYE
    wv_b = wv.astype(bf)
    maps = []
    for b in range(B):
        jp_b = np.concatenate([jp[b], _ROW1], 0)  # [65, HW]
        jg_b = np.concatenate([jg[b], _ROW1], 0)
        g_b = m @ jg_b  # [65, HW]
        jgt_b = np.concatenate([jg_b.T, np.zeros((HW, 1), np.float32)], 1)  # [HW, 66]
        maps.append(
            {
                "jp": jp_b.reshape(CE, N_QB, QB).astype(bf),
                "g": g_b.reshape(CE, N_KC, 128).astype(bf),
                "jgt": np.ascontiguousarray(
                    jgt_b.reshape(N_KC, 128, CT).transpose(1, 0, 2)
                ).astype(bf),
                "wv": wv_b,
                "ident": ident,
            }
        )
    return maps


def kernel(**inputs):
    nc = _get_module()
    in_maps = _prep_in_maps(inputs)
    res = run_bass_kernel_spmd(nc, in_maps, core_ids=list(range(N_CORES)))
    return np.stack(
        [res.results[b]["out"].reshape(D, H, W) for b in range(B)], axis=0
    )


# revision 8
# speedup vs baseline: 1.8135x; 1.0069x over previous
"""Fused cross-attention kernel for Trainium2, 8-way data-parallel over batch.

Low-rank decomposition: S = Q^T K = Jp'^T (Wq' Wk'^T) Jg' = Jp'^T G where
G = M Jg' is computed on host ([65, HW], M = Wq' Wk'^T is 65x65). This cuts
the score matmul contraction from 256 (d) to 65 (c), halving PE work, and
eliminates the Q/K projections entirely.

Value side uses the same trick: O = P V = P (Jg'^T Wv') = (P Jg'^T) Wv' = T Wv'
where T = P Jg'^T is [q, 66] (col 64 = softmax denominator via Jg' ones row,
col 65 = zero pad). Accumulating T costs 66 output columns per k-chunk instead
of 258 for direct P V.

Per core (one batch element):
  for each q-block (512 queries):
    for each k-chunk-pair (2 x 128 keys):
      S^T[k, 2, q] = G-chunk^T Jp'         (PE, bf16, PSUM 2 banks)
      E^T = exp(S^T / 16)                  (ScalarE, one 1024-wide activation)
      T[q, 4, 66] += E^T-subtile^T Jgt     (PE, bf16, 8 x 66-col matmuls)
    T -> SBUF, transpose via PE identity, O[q, 256] = T^T^T Wv'  (PE)
    out[q, :] = O * (1 / T[q, 64])         (DVE)

All matmul operands bf16 (1 cycle/col on PE regardless of width); PSUM
accumulation is fp32. ScalarE (exp) is the pacing engine at ~1us per
1024-element activation; PE runs at ~65% occupancy underneath it.
"""

import sys

sys.path.insert(0, "/opt/trn_rl_repo")

import numpy as np

import concourse.bacc as bacc
import concourse.mybir as mybir
import concourse.tile as tile
from concourse.bass_utils import run_bass_kernel_spmd

B, C, H, W = 8, 64, 64, 64
HW = H * W  # 4096
D = 256
CE = C + 1  # channels + ones row (bias folding)
CT = CE + 1  # T width: 65 channels + zero pad -> 66
N_CORES = 8
QB = 512  # queries per block
N_QB = HW // QB  # 8
N_KC = HW // 128  # 32 key chunks
N_CP = N_KC // 2  # 16 key-chunk pairs
F32 = mybir.dt.float32
BF16 = mybir.dt.bfloat16

_CACHE = {}


GROUPS = [3, 3, 3, 3, 3, 3, 3, 3, 3, 3, 2]  # k-chunks per exp activation (sum 32)


def build_module(reps: int = 1, st_bufs: int = 2, ep_bufs: int = 4, groups=None):
    if groups is None:
        groups = GROUPS
    assert sum(groups) == N_KC
    gmax = max(groups)
    nc = bacc.Bacc("TRN2", target_bir_lowering=False)
    jp_d = nc.dram_tensor("jp", [CE, N_QB, QB], BF16, kind="ExternalInput")
    g_d = nc.dram_tensor("g", [CE, N_KC, 128], BF16, kind="ExternalInput")
    jgt_d = nc.dram_tensor("jgt", [128, N_KC, CT], BF16, kind="ExternalInput")
    wv_d = nc.dram_tensor("wv", [CT, D], BF16, kind="ExternalInput")
    id_d = nc.dram_tensor("ident", [128, 128], F32, kind="ExternalInput")
    out_d = nc.dram_tensor("out", [HW, D], F32, kind="ExternalOutput")

    with tile.TileContext(nc) as tc:
        with tc.tile_pool(name="const", bufs=1) as const:
            jp_t = const.tile([CE, N_QB, QB], BF16, tag="jp")
            g_t = const.tile([CE, N_KC, 128], BF16, tag="g")
            jgt_t = const.tile([128, N_KC, CT], BF16, tag="jgt")
            wv_t = const.tile([CT, D], BF16, tag="wv")
            id_t = const.tile([128, 128], F32, tag="ident")

            # tiny dummy exp issued first: forces the ACT table load to happen
            # during the input-DMA window instead of on the critical path
            dummy = const.tile([1, 2], F32, tag="dummy")
            nc.vector.memset(dummy[:], 0.0)
            nc.scalar.activation(
                dummy[:], dummy[:], mybir.ActivationFunctionType.Exp
            )

            # critical-path inputs first on the sync queue: qb=0 queries and
            # the first score chunks; bulk/epilogue inputs on the gpsimd queue
            nc.sync.dma_start(jp_t[:, 0, :], jp_d[:, 0, :])
            for cq in range(4):
                ks = slice(8 * cq, 8 * cq + 8)
                nc.sync.dma_start(g_t[:, ks, :], g_d[:, ks, :])
            for cq in range(4):
                ks = slice(8 * cq, 8 * cq + 8)
                nc.gpsimd.dma_start(jgt_t[:, ks, :], jgt_d[:, ks, :])
            nc.gpsimd.dma_start(jp_t[:, 1:, :], jp_d[:, 1:, :])
            nc.gpsimd.dma_start(id_t[:], id_d[:])
            nc.gpsimd.dma_start(wv_t[:], wv_d[:])

            for _rep in range(reps):
                with (
                    tc.tile_pool(name="stp", bufs=st_bufs, space="PSUM") as stp,
                    tc.tile_pool(name="tp", bufs=1, space="PSUM") as tp,
                    tc.tile_pool(name="mx", bufs=1, space="PSUM") as mx,
                    tc.tile_pool(name="ep", bufs=ep_bufs) as ep,
                    tc.tile_pool(name="tsp", bufs=2) as tsp,
                    tc.tile_pool(name="ttsp", bufs=2) as ttsp,
                    tc.tile_pool(name="outp", bufs=3) as outp,
                    tc.tile_pool(name="lp", bufs=4) as lp,
                ):
                    for qb in range(N_QB):
                        t_ps = tp.tile([128, 4, CT], F32, tag="t", name=f"t_{qb}")
                        kc0 = 0
                        for gi, gn in enumerate(groups):
                            st = stp.tile(
                                [128, gmax, QB], F32, tag="st", name=f"st_{qb}_{gi}"
                            )
                            for c in range(gn):
                                nc.tensor.matmul(
                                    st[:, c, :],
                                    g_t[:, kc0 + c, :],
                                    jp_t[:, qb, :],
                                )
                            et = ep.tile([128, gmax, QB], BF16, tag="e")
                            nc.scalar.activation(
                                et[:, :gn, :],
                                st[:, :gn, :],
                                mybir.ActivationFunctionType.Exp,
                                scale=1.0 / 16.0,
                            )
                            for c in range(gn):
                                kc = kc0 + c
                                for j in range(4):
                                    # NOTE: start=True resets the WHOLE PSUM
                                    # bank, so only the first matmul of the
                                    # first group may use it — its reset
                                    # zero-fills the other 3 groups' regions.
                                    nc.tensor.matmul(
                                        t_ps[:, j, :],
                                        et[:, c, j * 128 : (j + 1) * 128],
                                        jgt_t[:, kc, :],
                                        start=(kc == 0 and j == 0),
                                        stop=(kc == N_KC - 1),
                                        skip_group_check=True,
                                    )
                            kc0 += gn
                        # epilogue: transpose T and project through Wv'
                        t_sb = tsp.tile([128, 4, CT], F32, tag="ts", name=f"ts_{qb}")
                        nc.vector.tensor_copy(t_sb[:], t_ps[:])
                        tt_ps = mx.tile(
                            [128, 4, 128], F32, tag="x", name=f"tt_{qb}"
                        )
                        for j in range(4):
                            nc.tensor.matmul(
                                tt_ps[:CT, j, :],
                                t_sb[:, j, :],
                                id_t[:],
                                is_transpose=True,
                                start=(j == 0),
                                stop=(j == 3),
                                skip_group_check=True,
                            )
                        tt_sb = ttsp.tile([128, 4, 128], BF16, tag="tts", name=f"tts_{qb}")
                        nc.vector.tensor_copy(tt_sb[:CT, :, :], tt_ps[:CT, :, :])
                        for j in range(4):
                            row = qb * 4 + j
                            o_ps = mx.tile(
                                [128, D],
                                F32,
                                tag="x",
                                name=f"o_{qb}_{j}",
                                padded_shape=[128, 4 * 128],
                            )
                            nc.tensor.matmul(
                                o_ps[:], tt_sb[:CT, j, :], wv_t[:]
                            )
                            linv = lp.tile([128, 1], F32, tag="l")
                            nc.vector.reciprocal(linv[:], t_sb[:, j, C : C + 1])
                            ot = outp.tile([128, D], F32, tag="ot")
                            nc.vector.tensor_scalar_mul(ot[:], o_ps[:], linv[:])
                            nc.sync.dma_start(
                                out_d[row * 128 : (row + 1) * 128, :], ot[:]
                            )

    nc.compile()
    return nc


def _get_module(reps: int = 1, **kw):
    key = (reps, tuple(sorted(kw.items())))
    if key not in _CACHE:
        _CACHE[key] = build_module(reps, **kw)
    return _CACHE[key]


_ROW1 = np.ones((1, HW), np.float32)
_EYE = np.eye(128, dtype=np.float32)


def _prep_in_maps(inputs):
    import ml_dtypes

    bf = ml_dtypes.bfloat16
    jp = np.asarray(inputs["Jp_embedding"], np.float32).reshape(B, C, HW)
    jg = np.asarray(inputs["Jg_embedding"], np.float32).reshape(B, C, HW)
    wq = np.concatenate(
        [np.asarray(inputs["Wq"], np.float32).T, np.asarray(inputs["bq"], np.float32)[None, :]], 0
    )
    wk = np.concatenate(
        [np.asarray(inputs["Wk"], np.float32).T, np.asarray(inputs["bk"], np.float32)[None, :]], 0
    )
    wv = np.concatenate(
        [
            np.asarray(inputs["Wv"], np.float32).T,
            np.asarray(inputs["bv"], np.float32)[None, :],
            np.zeros((1, D), np.float32),
        ],
        0,
    )
    m = wq @ wk.T  # [65, 65]
    ident = _EYE
    wv_b = wv.astype(bf)
    maps = []
    for b in range(B):
        jp_b = np.concatenate([jp[b], _ROW1], 0)  # [65, HW]
        jg_b = np.concatenate([jg[b], _ROW1], 0)
        g_b = m @ jg_b  # [65, HW]
        jgt_b = np.concatenate([jg_b.T, np.zeros((HW, 1), np.float32)], 1)  # [HW, 66]
        maps.append(
            {
                "jp": jp_b.reshape(CE, N_QB, QB).astype(bf),
                "g": g_b.reshape(CE, N_KC, 128).astype(bf),
                "jgt": np.ascontiguousarray(
                    jgt_b.reshape(N_KC, 128, CT).transpose(1, 0, 2)
                ).astype(bf),
                "wv": wv_b,
                "ident": ident,
            }
        )
    return maps


def kernel(**inputs):
    nc = _get_module()
    in_maps = _prep_in_maps(inputs)
    res = run_bass_kernel_spmd(nc, in_maps, core_ids=list(range(N_CORES)))
    return np.stack(
        [res.results[b]["out"].reshape(D, H, W) for b in range(B)], axis=0
    )
